# revision 26
# baseline (speedup 1.0000x reference)
"""Trainium2 Bass kernel for nn_Cal_adj_matrix (pyramid-pool adjacency).

Computes, per sample b:
    feature = x[b].reshape(C, M)                  # M = H*W = 9216
    pool    = pyramid_pool(x[b])                  # (C, 50), pools of size 1,2,3,6
    sim     = relu(feature^T @ pool / (B*C*H*W))  # (M, 50)
    total   = sim.sum(-1)                         # (M,)
    adj     = sim / (total^2 + 1e-6)              # (M, 50)

Sharding: data-parallel over batch; 32 samples -> 4 per core x 8 cores.

Perf structure (per core, 4 samples):
 - input streams fp32 over the SP HWDGE ring (the HBM roofline term);
   the full-input fp32->fp16 cast is split between the Act and Pool
   engines; pyramid pooling stage-1 reduces run on DVE from the fp32
   staging tiles in parallel with the casts.
 - matmuls keep feature stationary (output partition = m), accumulate
   the two 128-channel halves in PSUM, fp16 operands, RAW scale (the
   1/(B*C*H*W) divisor is applied in the tiny per-row stats chain so
   fp16 tiles stay in a safe numeric range).
 - output is written fp16 (halves the out-DMA bytes; rel-err budget is
   2e-2, fp16 adds ~5e-4) and the host casts back to fp32.
 - the LAST sample splits its c-half accumulation: the ch0 matmuls run
   while the ch1 half is still streaming in (drained to SBUF), so only
   the 72 ch1 matmuls + adds remain after the final input DMA. This
   halves the un-overlappable tail that otherwise dominates.
"""

import numpy as np

import concourse.bass as bass
import concourse.bacc as bacc
import concourse.mybir as mybir
import concourse.tile as tile
from concourse.bass_utils import run_bass_kernel_spmd

# Problem shape (hardcoded; kernel.py must be self-contained).
B, C, H, W = 32, 256, 96, 96
M = H * W            # 9216
N = 50               # 1 + 4 + 9 + 36 pyramid tokens
NCORES = 8
BS = B // NCORES     # 4 samples per core
DIV = float(B * C * H * W)  # reference's global divisor

FP32 = mybir.dt.float32
FP16 = mybir.dt.float16

# m-index mapping: m = p*72 + j  (p = partition, j = matmul index).
# This makes each sample's output one fully-contiguous DMA per chunk.
JN = M // 128        # 72 matmul column-groups per sample

BANK_J = 9           # matmul groups per PSUM bank (9*50=450 <= 512)
NBANK = JN // BANK_J  # 8 bank groups per sample


def build_nc(reps=1, xq_bufs=4, feat_bufs=4, outb_bufs=2, nq=4, nchunk=4,
             split_last=True, assist_q=(3,)):
    QH = H // nq         # h-rows per input chunk
    QM = QH * W          # elements per chunk
    nc = bacc.Bacc(
        "TRN2",
        target_bir_lowering=False,
        debug=False,
        enable_asserts=True,
        num_devices=NCORES,
    )
    x = nc.dram_tensor("x", [BS, C, H, W], FP32, kind="ExternalInput").ap()
    out = nc.dram_tensor("out", [BS, M, N], FP16, kind="ExternalOutput").ap()

    # mean scale factors for the pool values: 1/bin_elems (RAW sim scale:
    # the global 1/DIV is applied in the stats chain, not here, so fp16
    # tiles hold O(1) values)
    k1 = 1.0 / 9216.0
    k2 = 1.0 / 2304.0
    k3 = 1.0 / 1024.0
    k6 = 1.0 / 256.0

    with tile.TileContext(nc) as tc:
        with (
            tc.tile_pool(name="xq", bufs=xq_bufs) as xq_pool,
            tc.tile_pool(name="featfp", bufs=feat_bufs) as feat_pool,
            tc.tile_pool(name="r1", bufs=4) as r1_pool,
            tc.tile_pool(name="pools", bufs=8) as small_pool,
            tc.tile_pool(name="poolfp", bufs=4) as poolfp_pool,
            tc.tile_pool(name="outb", bufs=outb_bufs) as outb_pool,
            tc.tile_pool(name="simA", bufs=1) as simA_pool,
            tc.tile_pool(name="tmpS", bufs=2) as tmpS_pool,
            tc.tile_pool(name="stats", bufs=2) as stats_pool,
            tc.tile_pool(name="psum", bufs=8, space="PSUM") as psum_pool,
            nc.allow_low_precision(reason="fp16 staging within 2e-2 rel-err budget"),
        ):

            def load_half(s, ch):
                """DMA one 128-channel half + fp16 cast (Act/Pool split) +
                pyramid pooling -> (feat fp16 [128, M], pool fp16 [128, N])."""
                c0 = ch * 128
                fb = feat_pool.tile([128, M], FP16, tag="featfp")
                r1 = r1_pool.tile([128, 576], FP32, tag="r1")
                for q in range(nq):
                    h0 = q * QH
                    t32 = xq_pool.tile([128, QM], FP32, tag="xq")
                    src = x[s, c0:c0 + 128, h0:h0 + QH, :]
                    nc.sync.dma_start(out=t32[:], in_=src.rearrange("c h w -> c (h w)"))
                    # fp32 -> fp16 cast on Act (its queue holds only casts
                    # and relus, both ready-in-order; Pool's copy is 1.6x
                    # slower and its queue hosts the post chain)
                    nc.scalar.copy(fb[:, h0 * W:(h0 + QH) * W], t32[:])
                    # stage-1 pool: sum 16 contiguous w-elements into r1.
                    # DVE pair-add in fp16 (2x DVE mode) then an 8:1 reduce:
                    # ~1.8us/chunk vs 2.5us for a direct 16:1 fp32 reduce.
                    t8 = small_pool.tile([128, (QM // 16) * 8], FP16, tag="t8")
                    fv = fb[:, h0 * W:(h0 + QH) * W].rearrange(
                        "p (g two k) -> p g two k", two=2, k=8)
                    nc.vector.tensor_add(
                        t8[:, :].rearrange("p (g k) -> p g k", k=8),
                        fv[:, :, 0, :], fv[:, :, 1, :])
                    nc.vector.reduce_sum(
                        r1[:, h0 * 6:(h0 + QH) * 6],
                        t8[:, :].rearrange("p (g k) -> p g k", k=8),
                        axis=mybir.AxisListType.X,
                    )
                # stage-2: A[hb,wb] = 16x16 block sums.  r1 free idx = h*6+wb,
                # h = hb*16+hh  ->  idx = hb*96 + hh*6 + wb
                A = small_pool.tile([128, 36], FP32, tag="A")
                nc.vector.reduce_sum(
                    A[:, :],
                    r1[:, :576].rearrange("p (hb hh wb) -> p hb wb hh", hb=6, hh=16, wb=6),
                    axis=mybir.AxisListType.X,
                )
                # s=3 pools: 2x2 groups of A blocks
                Bt = small_pool.tile([128, 18], FP32, tag="B")  # [hb:6, wp:3]
                a2 = A[:, :36].rearrange("p (hb wp t) -> p t hb wp", hb=6, wp=3, t=2)
                nc.vector.tensor_add(Bt[:, :], a2[:, 0, :], a2[:, 1, :])
                s3raw = small_pool.tile([128, 9], FP32, tag="s3")
                b2 = Bt[:, :18].rearrange("p (hp t wp) -> p t hp wp", hp=3, t=2, wp=3)
                nc.vector.tensor_add(s3raw[:, :], b2[:, 0, :], b2[:, 1, :])
                # s=2 pools: 3x3 groups of A blocks
                Ct = small_pool.tile([128, 12], FP32, tag="C")  # [hb:6, wq:2]
                nc.vector.reduce_sum(
                    Ct[:, :],
                    A[:, :36].rearrange("p (hb wq wt) -> p (hb wq) wt", hb=6, wq=2, wt=3),
                    axis=mybir.AxisListType.X,
                )
                s2raw = small_pool.tile([128, 4], FP32, tag="s2")
                nc.vector.reduce_sum(
                    s2raw[:, :],
                    Ct[:, :12].rearrange("p (hq ht wq) -> p hq wq ht", hq=2, ht=3, wq=2),
                    axis=mybir.AxisListType.X,
                )
                # s=1 pool
                s1raw = small_pool.tile([128, 1], FP32, tag="s1")
                nc.vector.reduce_sum(s1raw[:, :], A[:, :36], axis=mybir.AxisListType.X)

                pb = poolfp_pool.tile([128, N], FP16, tag="poolfp")
                nc.vector.tensor_scalar_mul(pb[:, 0:1], s1raw[:, :], k1)
                nc.vector.tensor_scalar_mul(pb[:, 1:5], s2raw[:, :], k2)
                nc.vector.tensor_scalar_mul(pb[:, 5:14], s3raw[:, :], k3)
                nc.vector.tensor_scalar_mul(pb[:, 14:50], A[:, :], k6)
                return fb, pb

            def post_chunks(outb, s, nck=None):
                """Row stats + scale + fp16 output DMA, in nck chunks."""
                nck = nck or nchunk
                out_dram = out[s].rearrange("(p j) n -> p (j n)", p=128)
                JH = JN // nck
                half = JH * N
                for hf in range(nck):
                    sl = slice(hf * half, (hf + 1) * half)
                    ov = outb[:, sl].rearrange("p (j n) -> p j n", n=N)
                    # rowsum: Pool pre-adds n-halves (25+25) so the DVE
                    # reduce only reads half the elements
                    t25 = stats_pool.tile([128, JH * 25], FP16, tag="t25")
                    tv = t25[:, :].rearrange("p (j n) -> p j n", n=25)
                    nc.gpsimd.tensor_add(tv, ov[:, :, 0:25], ov[:, :, 25:50])
                    total = stats_pool.tile([128, JH], FP32, tag="total")
                    nc.vector.reduce_sum(total[:, :], tv, axis=mybir.AxisListType.X)
                    # scale = (1/DIV) / ((total/DIV)^2 + 1e-6):
                    # sq = total^2; sq = sq*(1/DIV^2)+1e-6; rc = recip * (1/DIV)
                    sq = stats_pool.tile([128, JH], FP32, tag="sq")
                    nc.vector.tensor_mul(sq[:, :], total[:, :], total[:, :])
                    nc.vector.tensor_scalar(
                        sq[:, :], sq[:, :], 1.0 / (DIV * DIV), 1e-6,
                        mybir.AluOpType.mult, mybir.AluOpType.add,
                    )
                    scale = stats_pool.tile([128, JH], FP32, tag="scale")
                    nc.vector.reciprocal(scale[:, :], sq[:, :])
                    rc = stats_pool.tile([128, JH], FP16, tag="rc")
                    nc.vector.tensor_scalar_mul(rc[:, :], scale[:, :], 1.0 / DIV)
                    nc.gpsimd.tensor_mul(
                        ov, ov, rc[:, :].unsqueeze(2).broadcast_to((128, JH, N)),
                    )
                    # contiguous output DMA (m = p*72 + j) issued from the
                    # Pool queue right after the mult that feeds it (zero
                    # issue-wait; never blocks the input ring or the casts)
                    nc.gpsimd.dma_start(out=out_dram[:, sl], in_=outb[:, sl])

            def finish_pools(A, small_pool_tiles=None):
                """stage-2 tail: s=3/2/1 pyramids + pb from a complete A."""
                Bt = small_pool.tile([128, 18], FP32, tag="B")
                a2 = A[:, :36].rearrange("p (hb wp t) -> p t hb wp", hb=6, wp=3, t=2)
                nc.vector.tensor_add(Bt[:, :], a2[:, 0, :], a2[:, 1, :])
                s3raw = small_pool.tile([128, 9], FP32, tag="s3")
                b2 = Bt[:, :18].rearrange("p (hp t wp) -> p t hp wp", hp=3, t=2, wp=3)
                nc.vector.tensor_add(s3raw[:, :], b2[:, 0, :], b2[:, 1, :])
                Ct = small_pool.tile([128, 12], FP32, tag="C")
                nc.vector.reduce_sum(
                    Ct[:, :],
                    A[:, :36].rearrange("p (hb wq wt) -> p (hb wq) wt", hb=6, wq=2, wt=3),
                    axis=mybir.AxisListType.X,
                )
                s2raw = small_pool.tile([128, 4], FP32, tag="s2")
                nc.vector.reduce_sum(
                    s2raw[:, :],
                    Ct[:, :12].rearrange("p (hq ht wq) -> p hq wq ht", hq=2, ht=3, wq=2),
                    axis=mybir.AxisListType.X,
                )
                s1raw = small_pool.tile([128, 1], FP32, tag="s1")
                nc.vector.reduce_sum(s1raw[:, :], A[:, :36], axis=mybir.AxisListType.X)
                pb = poolfp_pool.tile([128, N], FP16, tag="poolfp")
                nc.vector.tensor_scalar_mul(pb[:, 0:1], s1raw[:, :], k1)
                nc.vector.tensor_scalar_mul(pb[:, 1:5], s2raw[:, :], k2)
                nc.vector.tensor_scalar_mul(pb[:, 5:14], s3raw[:, :], k3)
                nc.vector.tensor_scalar_mul(pb[:, 14:50], A[:, :], k6)
                return pb

            def load_half_tail(s, ch):
                """Latency-optimized variant for the final half of the last
                sample: 6 DMA pieces of 16 h-rows; stage-1 reduces read the
                fp32 tiles directly (no cast dependency) and each piece
                completes its own A-row, so the pool vector is ready ~1.5us
                after the last input byte.  The fp16 cast (for the matmul
                stationary) still runs on Act in parallel."""
                c0 = ch * 128
                fb = feat_pool.tile([128, M], FP16, tag="featfp")
                A = small_pool.tile([128, 36], FP32, tag="A")
                for piece in range(6):
                    h0 = piece * 16
                    t32 = xq_pool.tile([128, 16 * W], FP32, tag="xq")
                    src = x[s, c0:c0 + 128, h0:h0 + 16, :]
                    nc.sync.dma_start(out=t32[:], in_=src.rearrange("c h w -> c (h w)"))
                    nc.scalar.copy(fb[:, h0 * W:(h0 + 16) * W], t32[:])
                    r1p = small_pool.tile([128, 96], FP32, tag="r1p")
                    nc.vector.reduce_sum(
                        r1p[:, :],
                        t32[:, :].rearrange("p (g k) -> p g k", k=16),
                        axis=mybir.AxisListType.X,
                    )
                    # this piece IS one 16-row block: finish its A row
                    nc.vector.reduce_sum(
                        A[:, piece * 6:(piece + 1) * 6],
                        r1p[:, :].rearrange("p (hh wb) -> p wb hh", hh=16, wb=6),
                        axis=mybir.AxisListType.X,
                    )
                pb = finish_pools(A)
                return fb, pb

            def drain_group(outb, out_dram, g, gs, nj=BANK_J):
                """Tail drain for one 9-j bank group: rowsum/stats/scale on
                DVE (idle during the tail), out-DMA issued from Act."""
                ov = outb[:, gs].rearrange("p (j n) -> p j n", n=N)
                total = stats_pool.tile([128, nj], FP32, tag="total")
                nc.vector.reduce_sum(total[:, :], ov, axis=mybir.AxisListType.X)
                sq = stats_pool.tile([128, nj], FP32, tag="sq")
                nc.vector.tensor_mul(sq[:, :], total[:, :], total[:, :])
                nc.vector.tensor_scalar(
                    sq[:, :], sq[:, :], 1.0 / (DIV * DIV), 1e-6,
                    mybir.AluOpType.mult, mybir.AluOpType.add,
                )
                scale = stats_pool.tile([128, nj], FP32, tag="scale")
                nc.vector.reciprocal(scale[:, :], sq[:, :])
                rc = stats_pool.tile([128, nj], FP16, tag="rc")
                nc.vector.tensor_scalar_mul(rc[:, :], scale[:, :], 1.0 / DIV)
                nc.vector.tensor_mul(
                    ov, ov, rc[:, :].unsqueeze(2).broadcast_to((128, nj, N)),
                )
                nc.scalar.dma_start(out=out_dram[:, gs], in_=outb[:, gs])

            def mm_phase(banks, fb, pb, first, last):
                """One c-half's matmul contributions.  Each PSUM bank is a
                single 18-matmul accumulation group (start on the first ch0
                matmul, stop on the last ch1 matmul), so the ch0 phase can
                run as soon as its half is resident -- for the last sample
                that overlaps the ch1 input DMA with no extra data movement.
                """
                for g in range(NBANK):
                    for k in range(BANK_J):
                        j = g * BANK_J + k
                        nc.tensor.matmul(
                            banks[g][:, k * N:(k + 1) * N],
                            fb[:, j:j + JN * 127 + 1:JN], pb[:, :],
                            start=(first and k == 0),
                            stop=(last and k == BANK_J - 1),
                        )

            def matmul_relu(fb0, pb0, fb1, pb1, s):
                """Matmuls + relu -> raw fp16 sim tile for one sample."""
                outb = outb_pool.tile([128, JN * N], FP16, tag="outb")
                for g in range(NBANK):
                    ps = psum_pool.tile([128, BANK_J * N], FP32, tag="ps")
                    for k in range(BANK_J):
                        j = g * BANK_J + k
                        nc.tensor.matmul(
                            ps[:, k * N:(k + 1) * N],
                            fb0[:, j:j + JN * 127 + 1:JN], pb0[:, :],
                            start=True, stop=False,
                        )
                        nc.tensor.matmul(
                            ps[:, k * N:(k + 1) * N],
                            fb1[:, j:j + JN * 127 + 1:JN], pb1[:, :],
                            start=False, stop=True,
                        )
                    nc.scalar.activation(
                        outb[:, g * BANK_J * N:(g + 1) * BANK_J * N],
                        ps[:, :], mybir.ActivationFunctionType.Relu,
                    )
                return outb

            def compute_sample(fb0, pb0, fb1, pb1, s):
                post_chunks(matmul_relu(fb0, pb0, fb1, pb1, s), s)

            # Software-pipelined emission: sample s-1's compute is emitted
            # AFTER sample s's loads, so each engine queue only holds ops
            # whose inputs are (nearly) ready -- no head-of-line blocking of
            # the casts/DMAs that feed the input stream.
            for rep in range(reps):
                pending = None
                for s in range(BS):
                    split = split_last and (s == BS - 1)
                    if not split:
                        h0 = load_half(s, 0)
                        h1 = load_half(s, 1)
                        if pending is not None:
                            compute_sample(*pending)
                        pending = (*h0, *h1, s)
                    else:
                        # Last sample: ch0 matmuls run during ch1's input
                        # DMA, drained raw to SBUF fp32; ch1 contributions
                        # are added back and relu'd.  Only the ch1 matmul
                        # stream + adds remain after the final input chunk.
                        fb0, pb0 = load_half(s, 0)
                        prev_outb = None
                        if pending is not None:
                            # matmuls+relus only: drains PSUM early and keeps
                            # the Act/DVE queues clear for the tail loader
                            *pp, ps_ = pending
                            prev_outb = (matmul_relu(*pending), ps_)
                            pending = None
                        banks = [psum_pool.tile([128, BANK_J * N], FP32, tag="ps",
                                                name=f"bankt{_g}") for _g in range(NBANK)]
                        mm_phase(banks, fb0, pb0, first=True, last=False)
                        fb1, pb1 = load_half_tail(s, 1)
                        if prev_outb is not None:
                            post_chunks(*prev_outb)
                        outb = outb_pool.tile([128, JN * N], FP16, tag="outb")
                        out_dram = out[s].rearrange("(p j) n -> p (j n)", p=128)
                        for g in range(NBANK):
                            gs = slice(g * BANK_J * N, (g + 1) * BANK_J * N)
                            for k in range(BANK_J):
                                j = g * BANK_J + k
                                nc.tensor.matmul(
                                    banks[g][:, k * N:(k + 1) * N],
                                    fb1[:, j:j + JN * 127 + 1:JN], pb1[:, :],
                                    start=False, stop=(k == BANK_J - 1),
                                )
                            nc.scalar.activation(
                                outb[:, gs], banks[g][:, :],
                                mybir.ActivationFunctionType.Relu,
                            )
                            if g % 2 == 1:
                                g2 = slice((g - 1) * BANK_J * N, (g + 1) * BANK_J * N)
                                drain_group(outb, out_dram, g, g2, nj=2 * BANK_J)
                if pending is not None:
                    compute_sample(*pending)

    nc.compile()
    return nc


_NC_CACHE = None


def kernel(**inputs) -> np.ndarray:
    global _NC_CACHE
    x = np.ascontiguousarray(np.asarray(inputs["x"], dtype=np.float32))
    assert x.shape == (B, C, H, W)
    if _NC_CACHE is None:
        _NC_CACHE = build_nc()
    nc = _NC_CACHE
    in_maps = [{"x": x[i * BS:(i + 1) * BS]} for i in range(NCORES)]
    res = run_bass_kernel_spmd(nc, in_maps, list(range(NCORES)))
    outs = [res.results[i]["out"] for i in range(NCORES)]
    return np.concatenate(outs, axis=0).astype(np.float32)


if __name__ == "__main__":
    xt = np.random.randn(B, C, H, W).astype(np.float32)
    y = kernel(x=xt)
    print(y.shape, y.dtype)


# revision 27
# speedup vs baseline: 1.0979x; 1.0979x over previous
"""Trainium2 Bass kernel for nn_Cal_adj_matrix (pyramid-pool adjacency).

Computes, per sample b:
    feature = x[b].reshape(C, M)                  # M = H*W = 9216
    pool    = pyramid_pool(x[b])                  # (C, 50), pools of size 1,2,3,6
    sim     = relu(feature^T @ pool / (B*C*H*W))  # (M, 50)
    total   = sim.sum(-1)                         # (M,)
    adj     = sim / (total^2 + 1e-6)              # (M, 50)

Sharding: data-parallel over batch; 32 samples -> 4 per core x 8 cores.

Perf structure (per core, 4 samples):
 - input streams fp32 over the SP HWDGE ring (the HBM roofline term);
   the full-input fp32->fp16 cast is split between the Act and Pool
   engines; pyramid pooling stage-1 reduces run on DVE from the fp32
   staging tiles in parallel with the casts.
 - matmuls keep feature stationary (output partition = m), accumulate
   the two 128-channel halves in PSUM, fp16 operands, RAW scale (the
   1/(B*C*H*W) divisor is applied in the tiny per-row stats chain so
   fp16 tiles stay in a safe numeric range).
 - output is written fp16 (halves the out-DMA bytes; rel-err budget is
   2e-2, fp16 adds ~5e-4) and the host casts back to fp32.
 - the LAST sample splits its c-half accumulation: each PSUM bank is a
   single 18-matmul accumulation group, so all 72 ch0 matmuls run while
   the ch1 half is still streaming in; only the ch1 matmul stream +
   per-bank relu/stats/DMA drain remain after the final input chunk.
   Its ch1 half uses a latency-optimized loader (stage-1 pooling straight
   off the fp32 staging tiles, per-piece A-rows) so the pool vector is
   ready ~1.5us after the last input byte.
"""

import numpy as np

import concourse.bass as bass
import concourse.bacc as bacc
import concourse.mybir as mybir
import concourse.tile as tile
from concourse.bass_utils import run_bass_kernel_spmd

# Problem shape (hardcoded; kernel.py must be self-contained).
B, C, H, W = 32, 256, 96, 96
M = H * W            # 9216
N = 50               # 1 + 4 + 9 + 36 pyramid tokens
NCORES = 8
BS = B // NCORES     # 4 samples per core
DIV = float(B * C * H * W)  # reference's global divisor

FP32 = mybir.dt.float32
FP16 = mybir.dt.float16

# m-index mapping: m = p*72 + j  (p = partition, j = matmul index).
# This makes each sample's output one fully-contiguous DMA per chunk.
JN = M // 128        # 72 matmul column-groups per sample

BANK_J = 9           # matmul groups per PSUM bank (9*50=450 <= 512)
NBANK = JN // BANK_J  # 8 bank groups per sample


def build_nc(reps=1, xq_bufs=4, feat_bufs=4, outb_bufs=2, nq=4, nchunk=4,
             split_last=True):
    QH = H // nq         # h-rows per input chunk
    QM = QH * W          # elements per chunk
    nc = bacc.Bacc(
        "TRN2",
        target_bir_lowering=False,
        debug=False,
        enable_asserts=True,
        num_devices=NCORES,
    )
    x = nc.dram_tensor("x", [BS, C, H, W], FP32, kind="ExternalInput").ap()
    out = nc.dram_tensor("out", [BS, M, N], FP16, kind="ExternalOutput").ap()

    # mean scale factors for the pool values: 1/bin_elems (RAW sim scale:
    # the global 1/DIV is applied in the stats chain, not here, so fp16
    # tiles hold O(1) values)
    k1 = 1.0 / 9216.0
    k2 = 1.0 / 2304.0
    k3 = 1.0 / 1024.0
    k6 = 1.0 / 256.0

    with tile.TileContext(nc) as tc:
        with (
            tc.tile_pool(name="xq", bufs=xq_bufs) as xq_pool,
            tc.tile_pool(name="featfp", bufs=feat_bufs) as feat_pool,
            tc.tile_pool(name="r1", bufs=4) as r1_pool,
            tc.tile_pool(name="pools", bufs=8) as small_pool,
            tc.tile_pool(name="poolfp", bufs=4) as poolfp_pool,
            tc.tile_pool(name="outb", bufs=outb_bufs) as outb_pool,
            tc.tile_pool(name="stats", bufs=2) as stats_pool,
            tc.tile_pool(name="psum", bufs=8, space="PSUM") as psum_pool,
            nc.allow_low_precision(reason="fp16 staging within 2e-2 rel-err budget"),
        ):

            def load_half(s, ch):
                """DMA one 128-channel half + fp16 cast (Act/Pool split) +
                pyramid pooling -> (feat fp16 [128, M], pool fp16 [128, N])."""
                c0 = ch * 128
                fb = feat_pool.tile([128, M], FP16, tag="featfp")
                r1 = r1_pool.tile([128, 576], FP32, tag="r1")
                for q in range(nq):
                    h0 = q * QH
                    t32 = xq_pool.tile([128, QM], FP32, tag="xq")
                    src = x[s, c0:c0 + 128, h0:h0 + QH, :]
                    nc.sync.dma_start(out=t32[:], in_=src.rearrange("c h w -> c (h w)"))
                    # fp32 -> fp16 cast on Act (its queue holds only casts
                    # and relus, both ready-in-order; Pool's copy is 1.6x
                    # slower and its queue hosts the post chain)
                    nc.scalar.copy(fb[:, h0 * W:(h0 + QH) * W], t32[:])
                    # stage-1 pool: sum 16 contiguous w-elements into r1.
                    # DVE pair-add in fp16 (2x DVE mode) then an 8:1 reduce:
                    # ~1.8us/chunk vs 2.5us for a direct 16:1 fp32 reduce.
                    t8 = small_pool.tile([128, (QM // 16) * 8], FP16, tag="t8")
                    fv = fb[:, h0 * W:(h0 + QH) * W].rearrange(
                        "p (g two k) -> p g two k", two=2, k=8)
                    nc.vector.tensor_add(
                        t8[:, :].rearrange("p (g k) -> p g k", k=8),
                        fv[:, :, 0, :], fv[:, :, 1, :])
                    nc.vector.reduce_sum(
                        r1[:, h0 * 6:(h0 + QH) * 6],
                        t8[:, :].rearrange("p (g k) -> p g k", k=8),
                        axis=mybir.AxisListType.X,
                    )
                # stage-2: A[hb,wb] = 16x16 block sums.  r1 free idx = h*6+wb,
                # h = hb*16+hh  ->  idx = hb*96 + hh*6 + wb
                A = small_pool.tile([128, 36], FP32, tag="A")
                nc.vector.reduce_sum(
                    A[:, :],
                    r1[:, :576].rearrange("p (hb hh wb) -> p hb wb hh", hb=6, hh=16, wb=6),
                    axis=mybir.AxisListType.X,
                )
                # s=3 pools: 2x2 groups of A blocks
                Bt = small_pool.tile([128, 18], FP32, tag="B")  # [hb:6, wp:3]
                a2 = A[:, :36].rearrange("p (hb wp t) -> p t hb wp", hb=6, wp=3, t=2)
                nc.vector.tensor_add(Bt[:, :], a2[:, 0, :], a2[:, 1, :])
                s3raw = small_pool.tile([128, 9], FP32, tag="s3")
                b2 = Bt[:, :18].rearrange("p (hp t wp) -> p t hp wp", hp=3, t=2, wp=3)
                nc.vector.tensor_add(s3raw[:, :], b2[:, 0, :], b2[:, 1, :])
                # s=2 pools: 3x3 groups of A blocks
                Ct = small_pool.tile([128, 12], FP32, tag="C")  # [hb:6, wq:2]
                nc.vector.reduce_sum(
                    Ct[:, :],
                    A[:, :36].rearrange("p (hb wq wt) -> p (hb wq) wt", hb=6, wq=2, wt=3),
                    axis=mybir.AxisListType.X,
                )
                s2raw = small_pool.tile([128, 4], FP32, tag="s2")
                nc.vector.reduce_sum(
                    s2raw[:, :],
                    Ct[:, :12].rearrange("p (hq ht wq) -> p hq wq ht", hq=2, ht=3, wq=2),
                    axis=mybir.AxisListType.X,
                )
                # s=1 pool
                s1raw = small_pool.tile([128, 1], FP32, tag="s1")
                nc.vector.reduce_sum(s1raw[:, :], A[:, :36], axis=mybir.AxisListType.X)

                pb = poolfp_pool.tile([128, N], FP16, tag="poolfp")
                nc.vector.tensor_scalar_mul(pb[:, 0:1], s1raw[:, :], k1)
                nc.vector.tensor_scalar_mul(pb[:, 1:5], s2raw[:, :], k2)
                nc.vector.tensor_scalar_mul(pb[:, 5:14], s3raw[:, :], k3)
                nc.vector.tensor_scalar_mul(pb[:, 14:50], A[:, :], k6)
                return fb, pb

            def post_chunks(outb, s, nck=None):
                """Row stats + scale + fp16 output DMA, in nck chunks."""
                nck = nck or nchunk
                out_dram = out[s].rearrange("(p j) n -> p (j n)", p=128)
                JH = JN // nck
                half = JH * N
                for hf in range(nck):
                    sl = slice(hf * half, (hf + 1) * half)
                    ov = outb[:, sl].rearrange("p (j n) -> p j n", n=N)
                    # rowsum: Pool pre-adds n-halves (25+25) so the DVE
                    # reduce only reads half the elements
                    t25 = stats_pool.tile([128, JH * 25], FP16, tag="t25")
                    tv = t25[:, :].rearrange("p (j n) -> p j n", n=25)
                    nc.gpsimd.tensor_add(tv, ov[:, :, 0:25], ov[:, :, 25:50])
                    total = stats_pool.tile([128, JH], FP32, tag="total")
                    nc.vector.reduce_sum(total[:, :], tv, axis=mybir.AxisListType.X)
                    # scale = (1/DIV) / ((total/DIV)^2 + 1e-6):
                    # sq = total^2; sq = sq*(1/DIV^2)+1e-6; rc = recip * (1/DIV)
                    sq = stats_pool.tile([128, JH], FP32, tag="sq")
                    nc.vector.tensor_mul(sq[:, :], total[:, :], total[:, :])
                    nc.vector.tensor_scalar(
                        sq[:, :], sq[:, :], 1.0 / (DIV * DIV), 1e-6,
                        mybir.AluOpType.mult, mybir.AluOpType.add,
                    )
                    scale = stats_pool.tile([128, JH], FP32, tag="scale")
                    nc.vector.reciprocal(scale[:, :], sq[:, :])
                    rc = stats_pool.tile([128, JH], FP16, tag="rc")
                    nc.vector.tensor_scalar_mul(rc[:, :], scale[:, :], 1.0 / DIV)
                    nc.gpsimd.tensor_mul(
                        ov, ov, rc[:, :].unsqueeze(2).broadcast_to((128, JH, N)),
                    )
                    # contiguous output DMA (m = p*72 + j) issued from the
                    # Pool queue right after the mult that feeds it (zero
                    # issue-wait; never blocks the input ring or the casts)
                    nc.gpsimd.dma_start(out=out_dram[:, sl], in_=outb[:, sl])

            def finish_pools(A, small_pool_tiles=None):
                """stage-2 tail: s=3/2/1 pyramids + pb from a complete A."""
                Bt = small_pool.tile([128, 18], FP32, tag="B")
                a2 = A[:, :36].rearrange("p (hb wp t) -> p t hb wp", hb=6, wp=3, t=2)
                nc.vector.tensor_add(Bt[:, :], a2[:, 0, :], a2[:, 1, :])
                s3raw = small_pool.tile([128, 9], FP32, tag="s3")
                b2 = Bt[:, :18].rearrange("p (hp t wp) -> p t hp wp", hp=3, t=2, wp=3)
                nc.vector.tensor_add(s3raw[:, :], b2[:, 0, :], b2[:, 1, :])
                Ct = small_pool.tile([128, 12], FP32, tag="C")
                nc.vector.reduce_sum(
                    Ct[:, :],
                    A[:, :36].rearrange("p (hb wq wt) -> p (hb wq) wt", hb=6, wq=2, wt=3),
                    axis=mybir.AxisListType.X,
                )
                s2raw = small_pool.tile([128, 4], FP32, tag="s2")
                nc.vector.reduce_sum(
                    s2raw[:, :],
                    Ct[:, :12].rearrange("p (hq ht wq) -> p hq wq ht", hq=2, ht=3, wq=2),
                    axis=mybir.AxisListType.X,
                )
                s1raw = small_pool.tile([128, 1], FP32, tag="s1")
                nc.vector.reduce_sum(s1raw[:, :], A[:, :36], axis=mybir.AxisListType.X)
                pb = poolfp_pool.tile([128, N], FP16, tag="poolfp")
                nc.vector.tensor_scalar_mul(pb[:, 0:1], s1raw[:, :], k1)
                nc.vector.tensor_scalar_mul(pb[:, 1:5], s2raw[:, :], k2)
                nc.vector.tensor_scalar_mul(pb[:, 5:14], s3raw[:, :], k3)
                nc.vector.tensor_scalar_mul(pb[:, 14:50], A[:, :], k6)
                return pb

            def load_half_tail(s, ch):
                """Latency-optimized variant for the final half of the last
                sample: 6 DMA pieces of 16 h-rows; stage-1 reduces read the
                fp32 tiles directly (no cast dependency) and each piece
                completes its own A-row, so the pool vector is ready ~1.5us
                after the last input byte.  The fp16 cast (for the matmul
                stationary) still runs on Act in parallel."""
                c0 = ch * 128
                fb = feat_pool.tile([128, M], FP16, tag="featfp")
                A = small_pool.tile([128, 36], FP32, tag="A")
                for piece in range(6):
                    h0 = piece * 16
                    t32 = xq_pool.tile([128, 16 * W], FP32, tag="xq")
                    src = x[s, c0:c0 + 128, h0:h0 + 16, :]
                    nc.sync.dma_start(out=t32[:], in_=src.rearrange("c h w -> c (h w)"))
                    nc.scalar.copy(fb[:, h0 * W:(h0 + 16) * W], t32[:])
                    r1p = small_pool.tile([128, 96], FP32, tag="r1p")
                    nc.vector.reduce_sum(
                        r1p[:, :],
                        t32[:, :].rearrange("p (g k) -> p g k", k=16),
                        axis=mybir.AxisListType.X,
                    )
                    # this piece IS one 16-row block: finish its A row
                    nc.vector.reduce_sum(
                        A[:, piece * 6:(piece + 1) * 6],
                        r1p[:, :].rearrange("p (hh wb) -> p wb hh", hh=16, wb=6),
                        axis=mybir.AxisListType.X,
                    )
                pb = finish_pools(A)
                return fb, pb

            def drain_group(outb, out_dram, g, gs, nj=BANK_J):
                """Tail drain for one 9-j bank group: rowsum/stats/scale on
                DVE (idle during the tail), out-DMA issued from Act."""
                ov = outb[:, gs].rearrange("p (j n) -> p j n", n=N)
                total = stats_pool.tile([128, nj], FP32, tag="total")
                nc.vector.reduce_sum(total[:, :], ov, axis=mybir.AxisListType.X)
                sq = stats_pool.tile([128, nj], FP32, tag="sq")
                nc.vector.tensor_mul(sq[:, :], total[:, :], total[:, :])
                nc.vector.tensor_scalar(
                    sq[:, :], sq[:, :], 1.0 / (DIV * DIV), 1e-6,
                    mybir.AluOpType.mult, mybir.AluOpType.add,
                )
                scale = stats_pool.tile([128, nj], FP32, tag="scale")
                nc.vector.reciprocal(scale[:, :], sq[:, :])
                rc = stats_pool.tile([128, nj], FP16, tag="rc")
                nc.vector.tensor_scalar_mul(rc[:, :], scale[:, :], 1.0 / DIV)
                nc.vector.tensor_mul(
                    ov, ov, rc[:, :].unsqueeze(2).broadcast_to((128, nj, N)),
                )
                nc.scalar.dma_start(out=out_dram[:, gs], in_=outb[:, gs])

            def mm_phase(banks, fb, pb, first, last):
                """One c-half's matmul contributions.  Each PSUM bank is a
                single 18-matmul accumulation group (start on the first ch0
                matmul, stop on the last ch1 matmul), so the ch0 phase can
                run as soon as its half is resident -- for the last sample
                that overlaps the ch1 input DMA with no extra data movement.
                """
                for g in range(NBANK):
                    for k in range(BANK_J):
                        j = g * BANK_J + k
                        nc.tensor.matmul(
                            banks[g][:, k * N:(k + 1) * N],
                            fb[:, j:j + JN * 127 + 1:JN], pb[:, :],
                            start=(first and k == 0),
                            stop=(last and k == BANK_J - 1),
                        )

            def matmul_relu(fb0, pb0, fb1, pb1, s):
                """Matmuls + relu -> raw fp16 sim tile for one sample."""
                outb = outb_pool.tile([128, JN * N], FP16, tag="outb")
                for g in range(NBANK):
                    ps = psum_pool.tile([128, BANK_J * N], FP32, tag="ps")
                    for k in range(BANK_J):
                        j = g * BANK_J + k
                        nc.tensor.matmul(
                            ps[:, k * N:(k + 1) * N],
                            fb0[:, j:j + JN * 127 + 1:JN], pb0[:, :],
                            start=True, stop=False,
                        )
                        nc.tensor.matmul(
                            ps[:, k * N:(k + 1) * N],
                            fb1[:, j:j + JN * 127 + 1:JN], pb1[:, :],
                            start=False, stop=True,
                        )
                    nc.scalar.activation(
                        outb[:, g * BANK_J * N:(g + 1) * BANK_J * N],
                        ps[:, :], mybir.ActivationFunctionType.Relu,
                    )
                return outb

            def compute_sample(fb0, pb0, fb1, pb1, s):
                post_chunks(matmul_relu(fb0, pb0, fb1, pb1, s), s)

            # Software-pipelined emission: sample s-1's compute is emitted
            # AFTER sample s's loads, so each engine queue only holds ops
            # whose inputs are (nearly) ready -- no head-of-line blocking of
            # the casts/DMAs that feed the input stream.
            for rep in range(reps):
                pending = None
                for s in range(BS):
                    split = split_last and (s == BS - 1)
                    if not split:
                        h0 = load_half(s, 0)
                        h1 = load_half(s, 1)
                        if pending is not None:
                            compute_sample(*pending)
                        pending = (*h0, *h1, s)
                    else:
                        # Last sample: ch0 matmuls run during ch1's input
                        # DMA, drained raw to SBUF fp32; ch1 contributions
                        # are added back and relu'd.  Only the ch1 matmul
                        # stream + adds remain after the final input chunk.
                        fb0, pb0 = load_half(s, 0)
                        prev_outb = None
                        if pending is not None:
                            # matmuls+relus only: drains PSUM early and keeps
                            # the Act/DVE queues clear for the tail loader
                            *pp, ps_ = pending
                            prev_outb = (matmul_relu(*pending), ps_)
                            pending = None
                        banks = [psum_pool.tile([128, BANK_J * N], FP32, tag="ps",
                                                name=f"bankt{_g}") for _g in range(NBANK)]
                        mm_phase(banks, fb0, pb0, first=True, last=False)
                        fb1, pb1 = load_half_tail(s, 1)
                        if prev_outb is not None:
                            post_chunks(*prev_outb)
                        outb = outb_pool.tile([128, JN * N], FP16, tag="outb")
                        out_dram = out[s].rearrange("(p j) n -> p (j n)", p=128)
                        for g in range(NBANK):
                            gs = slice(g * BANK_J * N, (g + 1) * BANK_J * N)
                            for k in range(BANK_J):
                                j = g * BANK_J + k
                                nc.tensor.matmul(
                                    banks[g][:, k * N:(k + 1) * N],
                                    fb1[:, j:j + JN * 127 + 1:JN], pb1[:, :],
                                    start=False, stop=(k == BANK_J - 1),
                                )
                            nc.scalar.activation(
                                outb[:, gs], banks[g][:, :],
                                mybir.ActivationFunctionType.Relu,
                            )
                            if g % 2 == 1:
                                g2 = slice((g - 1) * BANK_J * N, (g + 1) * BANK_J * N)
                                drain_group(outb, out_dram, g, g2, nj=2 * BANK_J)
                if pending is not None:
                    compute_sample(*pending)

    nc.compile()
    return nc


_NC_CACHE = None


def kernel(**inputs) -> np.ndarray:
    global _NC_CACHE
    x = np.ascontiguousarray(np.asarray(inputs["x"], dtype=np.float32))
    assert x.shape == (B, C, H, W)
    if _NC_CACHE is None:
        _NC_CACHE = build_nc()
    nc = _NC_CACHE
    in_maps = [{"x": x[i * BS:(i + 1) * BS]} for i in range(NCORES)]
    res = run_bass_kernel_spmd(nc, in_maps, list(range(NCORES)))
    outs = [res.results[i]["out"] for i in range(NCORES)]
    return np.concatenate(outs, axis=0).astype(np.float32)


if __name__ == "__main__":
    xt = np.random.randn(B, C, H, W).astype(np.float32)
    y = kernel(x=xt)
    print(y.shape, y.dtype)


# revision 31
# speedup vs baseline: 1.1647x; 1.0608x over previous
"""Trainium2 Bass kernel for nn_Cal_adj_matrix (pyramid-pool adjacency).

Computes, per sample b:
    feature = x[b].reshape(C, M)                  # M = H*W = 9216
    pool    = pyramid_pool(x[b])                  # (C, 50), pools of size 1,2,3,6
    sim     = relu(feature^T @ pool / (B*C*H*W))  # (M, 50)
    total   = sim.sum(-1)                         # (M,)
    adj     = sim / (total^2 + 1e-6)              # (M, 50)

Sharding: data-parallel over batch; 32 samples -> 4 per core x 8 cores.

Perf structure (per core, 4 samples):
 - input streams fp32 over the SP HWDGE ring (the HBM roofline term);
   the full-input fp32->fp16 cast is split between the Act and Pool
   engines; pyramid pooling stage-1 reduces run on DVE from the fp32
   staging tiles in parallel with the casts.
 - matmuls keep feature stationary (output partition = m), accumulate
   the two 128-channel halves in PSUM, fp16 operands, RAW scale (the
   1/(B*C*H*W) divisor is applied in the tiny per-row stats chain so
   fp16 tiles stay in a safe numeric range).
 - output is written fp16 (halves the out-DMA bytes; rel-err budget is
   2e-2, fp16 adds ~5e-4) and the host casts back to fp32.
 - the LAST sample splits its c-half accumulation: each PSUM bank is a
   single 18-matmul accumulation group, so all 72 ch0 matmuls run while
   the ch1 half is still streaming in; only the ch1 matmul stream +
   per-bank relu/stats/DMA drain remain after the final input chunk.
   Its ch1 half uses a latency-optimized loader (stage-1 pooling straight
   off the fp32 staging tiles, per-piece A-rows) so the pool vector is
   ready ~1.5us after the last input byte.
"""

import numpy as np

import concourse.bass as bass
import concourse.bacc as bacc
import concourse.mybir as mybir
import concourse.tile as tile
from concourse.bass_utils import run_bass_kernel_spmd

# Problem shape (hardcoded; kernel.py must be self-contained).
B, C, H, W = 32, 256, 96, 96
M = H * W            # 9216
N = 50               # 1 + 4 + 9 + 36 pyramid tokens
NCORES = 8
BS = B // NCORES     # 4 samples per core
DIV = float(B * C * H * W)  # reference's global divisor

FP32 = mybir.dt.float32
FP16 = mybir.dt.float16

# m-index mapping: m = p*72 + j  (p = partition, j = matmul index).
# This makes each sample's output one fully-contiguous DMA per chunk.
JN = M // 128        # 72 matmul column-groups per sample

BANK_J = 9           # matmul groups per PSUM bank (9*50=450 <= 512)
NBANK = JN // BANK_J  # 8 bank groups per sample


def build_nc(reps=1, xq_bufs=3, feat_bufs=4, outb_bufs=2, nq=2, nchunk=4,
             split_last=True):
    QH = H // nq         # h-rows per input chunk
    QM = QH * W          # elements per chunk
    nc = bacc.Bacc(
        "TRN2",
        target_bir_lowering=False,
        debug=False,
        enable_asserts=True,
        num_devices=NCORES,
    )
    x = nc.dram_tensor("x", [BS, C, H, W], FP32, kind="ExternalInput").ap()
    out = nc.dram_tensor("out", [BS, M, N], FP16, kind="ExternalOutput").ap()

    # mean scale factors for the pool values: 1/bin_elems (RAW sim scale:
    # the global 1/DIV is applied in the stats chain, not here, so fp16
    # tiles hold O(1) values)
    k1 = 1.0 / 9216.0
    k2 = 1.0 / 2304.0
    k3 = 1.0 / 1024.0
    k6 = 1.0 / 256.0

    with tile.TileContext(nc) as tc:
        with (
            tc.tile_pool(name="xq", bufs=xq_bufs) as xq_pool,
            tc.tile_pool(name="featfp", bufs=feat_bufs) as feat_pool,
            tc.tile_pool(name="r1", bufs=4) as r1_pool,
            tc.tile_pool(name="pools", bufs=8) as small_pool,
            tc.tile_pool(name="poolfp", bufs=4) as poolfp_pool,
            tc.tile_pool(name="outb", bufs=outb_bufs) as outb_pool,
            tc.tile_pool(name="stats", bufs=2) as stats_pool,
            tc.tile_pool(name="psum", bufs=8, space="PSUM") as psum_pool,
            nc.allow_low_precision(reason="fp16 staging within 2e-2 rel-err budget"),
        ):

            def load_half(s, ch):
                """DMA one 128-channel half + fp16 cast (Act/Pool split) +
                pyramid pooling -> (feat fp16 [128, M], pool fp16 [128, N])."""
                c0 = ch * 128
                fb = feat_pool.tile([128, M], FP16, tag="featfp")
                r1 = r1_pool.tile([128, 576], FP32, tag="r1")
                for q in range(nq):
                    h0 = q * QH
                    t32 = xq_pool.tile([128, QM], FP32, tag="xq")
                    src = x[s, c0:c0 + 128, h0:h0 + QH, :]
                    nc.sync.dma_start(out=t32[:], in_=src.rearrange("c h w -> c (h w)"))
                    # fp32 -> fp16 cast on Act (its queue holds only casts
                    # and relus, both ready-in-order; Pool's copy is 1.6x
                    # slower and its queue hosts the post chain)
                    nc.scalar.copy(fb[:, h0 * W:(h0 + QH) * W], t32[:])
                    # stage-1 pool: sum 16 contiguous w-elements into r1.
                    # DVE pair-add in fp16 (2x DVE mode) then an 8:1 reduce:
                    # ~1.8us/chunk vs 2.5us for a direct 16:1 fp32 reduce.
                    t8 = small_pool.tile([128, (QM // 16) * 8], FP16, tag="t8")
                    fv = fb[:, h0 * W:(h0 + QH) * W].rearrange(
                        "p (g two k) -> p g two k", two=2, k=8)
                    nc.vector.tensor_add(
                        t8[:, :].rearrange("p (g k) -> p g k", k=8),
                        fv[:, :, 0, :], fv[:, :, 1, :])
                    nc.vector.reduce_sum(
                        r1[:, h0 * 6:(h0 + QH) * 6],
                        t8[:, :].rearrange("p (g k) -> p g k", k=8),
                        axis=mybir.AxisListType.X,
                    )
                # stage-2: A[hb,wb] = 16x16 block sums.  r1 free idx = h*6+wb,
                # h = hb*16+hh  ->  idx = hb*96 + hh*6 + wb
                A = small_pool.tile([128, 36], FP32, tag="A")
                nc.vector.reduce_sum(
                    A[:, :],
                    r1[:, :576].rearrange("p (hb hh wb) -> p hb wb hh", hb=6, hh=16, wb=6),
                    axis=mybir.AxisListType.X,
                )
                # s=3 pools: 2x2 groups of A blocks
                Bt = small_pool.tile([128, 18], FP32, tag="B")  # [hb:6, wp:3]
                a2 = A[:, :36].rearrange("p (hb wp t) -> p t hb wp", hb=6, wp=3, t=2)
                nc.vector.tensor_add(Bt[:, :], a2[:, 0, :], a2[:, 1, :])
                s3raw = small_pool.tile([128, 9], FP32, tag="s3")
                b2 = Bt[:, :18].rearrange("p (hp t wp) -> p t hp wp", hp=3, t=2, wp=3)
                nc.vector.tensor_add(s3raw[:, :], b2[:, 0, :], b2[:, 1, :])
                # s=2 pools: 3x3 groups of A blocks
                Ct = small_pool.tile([128, 12], FP32, tag="C")  # [hb:6, wq:2]
                nc.vector.reduce_sum(
                    Ct[:, :],
                    A[:, :36].rearrange("p (hb wq wt) -> p (hb wq) wt", hb=6, wq=2, wt=3),
                    axis=mybir.AxisListType.X,
                )
                s2raw = small_pool.tile([128, 4], FP32, tag="s2")
                nc.vector.reduce_sum(
                    s2raw[:, :],
                    Ct[:, :12].rearrange("p (hq ht wq) -> p hq wq ht", hq=2, ht=3, wq=2),
                    axis=mybir.AxisListType.X,
                )
                # s=1 pool
                s1raw = small_pool.tile([128, 1], FP32, tag="s1")
                nc.vector.reduce_sum(s1raw[:, :], A[:, :36], axis=mybir.AxisListType.X)

                pb = poolfp_pool.tile([128, N], FP16, tag="poolfp")
                nc.vector.tensor_scalar_mul(pb[:, 0:1], s1raw[:, :], k1)
                nc.vector.tensor_scalar_mul(pb[:, 1:5], s2raw[:, :], k2)
                nc.vector.tensor_scalar_mul(pb[:, 5:14], s3raw[:, :], k3)
                nc.vector.tensor_scalar_mul(pb[:, 14:50], A[:, :], k6)
                return fb, pb

            def post_chunks(outb, s, nck=None):
                """Row stats + scale + fp16 output DMA, in nck chunks."""
                nck = nck or nchunk
                out_dram = out[s].rearrange("(p j) n -> p (j n)", p=128)
                JH = JN // nck
                half = JH * N
                for hf in range(nck):
                    sl = slice(hf * half, (hf + 1) * half)
                    ov = outb[:, sl].rearrange("p (j n) -> p j n", n=N)
                    # rowsum: Pool pre-adds n-halves (25+25) so the DVE
                    # reduce only reads half the elements
                    t25 = stats_pool.tile([128, JH * 25], FP16, tag="t25")
                    tv = t25[:, :].rearrange("p (j n) -> p j n", n=25)
                    nc.gpsimd.tensor_add(tv, ov[:, :, 0:25], ov[:, :, 25:50])
                    total = stats_pool.tile([128, JH], FP32, tag="total")
                    nc.vector.reduce_sum(total[:, :], tv, axis=mybir.AxisListType.X)
                    # scale = (1/DIV) / ((total/DIV)^2 + 1e-6):
                    # sq = total^2; sq = sq*(1/DIV^2)+1e-6; rc = recip * (1/DIV)
                    sq = stats_pool.tile([128, JH], FP32, tag="sq")
                    nc.vector.tensor_mul(sq[:, :], total[:, :], total[:, :])
                    nc.vector.tensor_scalar(
                        sq[:, :], sq[:, :], 1.0 / (DIV * DIV), 1e-6,
                        mybir.AluOpType.mult, mybir.AluOpType.add,
                    )
                    scale = stats_pool.tile([128, JH], FP32, tag="scale")
                    nc.vector.reciprocal(scale[:, :], sq[:, :])
                    rc = stats_pool.tile([128, JH], FP16, tag="rc")
                    nc.vector.tensor_scalar_mul(rc[:, :], scale[:, :], 1.0 / DIV)
                    nc.gpsimd.tensor_mul(
                        ov, ov, rc[:, :].unsqueeze(2).broadcast_to((128, JH, N)),
                    )
                    # contiguous output DMA (m = p*72 + j) issued from the
                    # Pool queue right after the mult that feeds it (zero
                    # issue-wait; never blocks the input ring or the casts)
                    nc.gpsimd.dma_start(out=out_dram[:, sl], in_=outb[:, sl])

            def finish_pools(A, small_pool_tiles=None):
                """stage-2 tail: s=3/2/1 pyramids + pb from a complete A."""
                Bt = small_pool.tile([128, 18], FP32, tag="B")
                a2 = A[:, :36].rearrange("p (hb wp t) -> p t hb wp", hb=6, wp=3, t=2)
                nc.vector.tensor_add(Bt[:, :], a2[:, 0, :], a2[:, 1, :])
                s3raw = small_pool.tile([128, 9], FP32, tag="s3")
                b2 = Bt[:, :18].rearrange("p (hp t wp) -> p t hp wp", hp=3, t=2, wp=3)
                nc.vector.tensor_add(s3raw[:, :], b2[:, 0, :], b2[:, 1, :])
                Ct = small_pool.tile([128, 12], FP32, tag="C")
                nc.vector.reduce_sum(
                    Ct[:, :],
                    A[:, :36].rearrange("p (hb wq wt) -> p (hb wq) wt", hb=6, wq=2, wt=3),
                    axis=mybir.AxisListType.X,
                )
                s2raw = small_pool.tile([128, 4], FP32, tag="s2")
                nc.vector.reduce_sum(
                    s2raw[:, :],
                    Ct[:, :12].rearrange("p (hq ht wq) -> p hq wq ht", hq=2, ht=3, wq=2),
                    axis=mybir.AxisListType.X,
                )
                s1raw = small_pool.tile([128, 1], FP32, tag="s1")
                nc.vector.reduce_sum(s1raw[:, :], A[:, :36], axis=mybir.AxisListType.X)
                pb = poolfp_pool.tile([128, N], FP16, tag="poolfp")
                nc.vector.tensor_scalar_mul(pb[:, 0:1], s1raw[:, :], k1)
                nc.vector.tensor_scalar_mul(pb[:, 1:5], s2raw[:, :], k2)
                nc.vector.tensor_scalar_mul(pb[:, 5:14], s3raw[:, :], k3)
                nc.vector.tensor_scalar_mul(pb[:, 14:50], A[:, :], k6)
                return pb

            def load_half_tail(s, ch):
                """Latency-optimized variant for the final half of the last
                sample: 6 DMA pieces of 16 h-rows; stage-1 reduces read the
                fp32 tiles directly (no cast dependency) and each piece
                completes its own A-row, so the pool vector is ready ~1.5us
                after the last input byte.  The fp16 cast (for the matmul
                stationary) still runs on Act in parallel."""
                c0 = ch * 128
                fb = feat_pool.tile([128, M], FP16, tag="featfp")
                A = small_pool.tile([128, 36], FP32, tag="A")
                for piece in range(6):
                    h0 = piece * 16
                    t32 = xq_pool.tile([128, 16 * W], FP32, tag="xq")
                    src = x[s, c0:c0 + 128, h0:h0 + 16, :]
                    nc.sync.dma_start(out=t32[:], in_=src.rearrange("c h w -> c (h w)"))
                    nc.scalar.copy(fb[:, h0 * W:(h0 + 16) * W], t32[:])
                    r1p = small_pool.tile([128, 96], FP32, tag="r1p")
                    nc.vector.reduce_sum(
                        r1p[:, :],
                        t32[:, :].rearrange("p (g k) -> p g k", k=16),
                        axis=mybir.AxisListType.X,
                    )
                    # this piece IS one 16-row block: finish its A row
                    nc.vector.reduce_sum(
                        A[:, piece * 6:(piece + 1) * 6],
                        r1p[:, :].rearrange("p (hh wb) -> p wb hh", hh=16, wb=6),
                        axis=mybir.AxisListType.X,
                    )
                pb = finish_pools(A)
                return fb, pb

            def drain_group(outb, out_dram, g, gs, nj=BANK_J):
                """Tail drain for one 9-j bank group: rowsum/stats/scale on
                DVE (idle during the tail), out-DMA issued from Act."""
                ov = outb[:, gs].rearrange("p (j n) -> p j n", n=N)
                total = stats_pool.tile([128, nj], FP32, tag="total")
                nc.vector.reduce_sum(total[:, :], ov, axis=mybir.AxisListType.X)
                sq = stats_pool.tile([128, nj], FP32, tag="sq")
                nc.vector.tensor_mul(sq[:, :], total[:, :], total[:, :])
                nc.vector.tensor_scalar(
                    sq[:, :], sq[:, :], 1.0 / (DIV * DIV), 1e-6,
                    mybir.AluOpType.mult, mybir.AluOpType.add,
                )
                scale = stats_pool.tile([128, nj], FP32, tag="scale")
                nc.vector.reciprocal(scale[:, :], sq[:, :])
                rc = stats_pool.tile([128, nj], FP16, tag="rc")
                nc.vector.tensor_scalar_mul(rc[:, :], scale[:, :], 1.0 / DIV)
                nc.vector.tensor_mul(
                    ov, ov, rc[:, :].unsqueeze(2).broadcast_to((128, nj, N)),
                )
                nc.scalar.dma_start(out=out_dram[:, gs], in_=outb[:, gs])

            def mm_phase(banks, fb, pb, first, last):
                """One c-half's matmul contributions.  Each PSUM bank is a
                single 18-matmul accumulation group (start on the first ch0
                matmul, stop on the last ch1 matmul), so the ch0 phase can
                run as soon as its half is resident -- for the last sample
                that overlaps the ch1 input DMA with no extra data movement.
                """
                for g in range(NBANK):
                    for k in range(BANK_J):
                        j = g * BANK_J + k
                        nc.tensor.matmul(
                            banks[g][:, k * N:(k + 1) * N],
                            fb[:, j:j + JN * 127 + 1:JN], pb[:, :],
                            start=(first and k == 0),
                            stop=(last and k == BANK_J - 1),
                        )

            def matmul_relu(fb0, pb0, fb1, pb1, s):
                """Matmuls + relu -> raw fp16 sim tile for one sample."""
                outb = outb_pool.tile([128, JN * N], FP16, tag="outb")
                for g in range(NBANK):
                    ps = psum_pool.tile([128, BANK_J * N], FP32, tag="ps")
                    for k in range(BANK_J):
                        j = g * BANK_J + k
                        nc.tensor.matmul(
                            ps[:, k * N:(k + 1) * N],
                            fb0[:, j:j + JN * 127 + 1:JN], pb0[:, :],
                            start=True, stop=False,
                        )
                        nc.tensor.matmul(
                            ps[:, k * N:(k + 1) * N],
                            fb1[:, j:j + JN * 127 + 1:JN], pb1[:, :],
                            start=False, stop=True,
                        )
                    nc.scalar.activation(
                        outb[:, g * BANK_J * N:(g + 1) * BANK_J * N],
                        ps[:, :], mybir.ActivationFunctionType.Relu,
                    )
                return outb

            def compute_sample(fb0, pb0, fb1, pb1, s):
                post_chunks(matmul_relu(fb0, pb0, fb1, pb1, s), s)

            # Software-pipelined emission: sample s-1's compute is emitted
            # AFTER sample s's loads, so each engine queue only holds ops
            # whose inputs are (nearly) ready -- no head-of-line blocking of
            # the casts/DMAs that feed the input stream.
            for rep in range(reps):
                pending = None
                for s in range(BS):
                    split = split_last and (s == BS - 1)
                    if not split:
                        h0 = load_half(s, 0)
                        h1 = load_half(s, 1)
                        if pending is not None:
                            compute_sample(*pending)
                        pending = (*h0, *h1, s)
                    else:
                        # Last sample: ch0 matmuls run during ch1's input
                        # DMA, drained raw to SBUF fp32; ch1 contributions
                        # are added back and relu'd.  Only the ch1 matmul
                        # stream + adds remain after the final input chunk.
                        fb0, pb0 = load_half(s, 0)
                        prev_outb = None
                        if pending is not None:
                            # matmuls+relus only: drains PSUM early and keeps
                            # the Act/DVE queues clear for the tail loader
                            *pp, ps_ = pending
                            prev_outb = (matmul_relu(*pending), ps_)
                            pending = None
                        banks = [psum_pool.tile([128, BANK_J * N], FP32, tag="ps",
                                                name=f"bankt{_g}") for _g in range(NBANK)]
                        mm_phase(banks, fb0, pb0, first=True, last=False)
                        fb1, pb1 = load_half_tail(s, 1)
                        if prev_outb is not None:
                            post_chunks(*prev_outb)
                        outb = outb_pool.tile([128, JN * N], FP16, tag="outb")
                        out_dram = out[s].rearrange("(p j) n -> p (j n)", p=128)
                        for g in range(NBANK):
                            gs = slice(g * BANK_J * N, (g + 1) * BANK_J * N)
                            for k in range(BANK_J):
                                j = g * BANK_J + k
                                nc.tensor.matmul(
                                    banks[g][:, k * N:(k + 1) * N],
                                    fb1[:, j:j + JN * 127 + 1:JN], pb1[:, :],
                                    start=False, stop=(k == BANK_J - 1),
                                )
                            nc.scalar.activation(
                                outb[:, gs], banks[g][:, :],
                                mybir.ActivationFunctionType.Relu,
                            )
                            if g % 2 == 1:
                                g2 = slice((g - 1) * BANK_J * N, (g + 1) * BANK_J * N)
                                drain_group(outb, out_dram, g, g2, nj=2 * BANK_J)
                if pending is not None:
                    compute_sample(*pending)

    nc.compile()
    return nc


_NC_CACHE = None


def kernel(**inputs) -> np.ndarray:
    global _NC_CACHE
    x = np.ascontiguousarray(np.asarray(inputs["x"], dtype=np.float32))
    assert x.shape == (B, C, H, W)
    if _NC_CACHE is None:
        _NC_CACHE = build_nc()
    nc = _NC_CACHE
    in_maps = [{"x": x[i * BS:(i + 1) * BS]} for i in range(NCORES)]
    res = run_bass_kernel_spmd(nc, in_maps, list(range(NCORES)))
    outs = [res.results[i]["out"] for i in range(NCORES)]
    return np.concatenate(outs, axis=0).astype(np.float32)


if __name__ == "__main__":
    xt = np.random.randn(B, C, H, W).astype(np.float32)
    y = kernel(x=xt)
    print(y.shape, y.dtype)
